# revision 31
# baseline (speedup 1.0000x reference)
"""Trainium2 Bass kernel for 3D windowed (3x3x3) per-channel softmax attention.

Problem (hardcoded): x (1,32,24,48,64) f32; Wq/Wk/Wv (48,32); rel_* (16,...,3).
  q = Wq@x ; kf/vf = Wk/Wv @ pad(x) ; per (c,voxel): softmax over the 27
  window taps of q*(k_win+rel), then weighted sum of v_win.

Strategy (v2):
  - Shard D=24 across 8 cores (3 output d-slices each + 1-voxel halo,
    zero-padded on host). SPMD, no collectives.
  - Rows r = (s, c_sub): 8 H-blocks x 16 channels = 128 partitions/pass,
    3 passes = one channel GROUP of 16 per pass. Within a pass every
    channel shares the same rel axis (ch 0-15: rel_d varies over wj,
    16-31: rel_h over dj, 32-47: rel_w over hj), so
      exp(q*(k+rel_j)) = exp(q*k_j) * F_a,   F_a = exp(q*rel_a)
    factors with only 3 F planes per pass.
  - qk logits via plain tensor_tensor (2x DVE mode; the baseline's
    scalar_tensor_tensor runs at 1x), batched 3 taps per op via window APs.
  - exp on ACT (in-place over the logit planes).
  - Per-axis sums S_a = sum_{j in a} e_j and T_a = sum_{j in a} e_j*v_j
    either on PE (identity-matmul PSUM accumulation, Pool evicts) or on
    DVE (pairwise trees) - configurable via RED_DEN/RED_NUM.
  - den = sum_a F_a*S_a, num = sum_a F_a*T_a, out = num * recip(den).
  - No DRAM bounce: projections go PSUM -> SBUF stage -> SBUF-SBUF DMA
    gather into per-pass row tiles.
"""

import sys

sys.path.insert(0, "/opt/trn_rl_repo")

import numpy as np

import concourse.bass as bass
import concourse.bacc as bacc
import concourse.mybir as mybir
import concourse.tile as tile
from concourse.bass_utils import run_bass_kernel_spmd

# ---- problem constants (hardcoded per contract) ----
B, CIN, D, H, W = 1, 32, 24, 48, 64
COUT, K, C3 = 48, 3, 16
NCORES = 8
DLOC = D // NCORES            # 3 output d-slices per core
DP = DLOC + 2                 # 5 padded d-planes per core
NS = 8                        # H-blocks per core
HB = H // NS                  # 6 output rows per block
HBP = HB + 2                  # 8 padded rows per block
WP = W + 2                    # 66
BLK = DP * HBP * WP           # 2640 padded voxels per block
SL = HBP * WP                 # 528: one padded d-plane
FL = (HB - 1) * WP + W        # 394-elem flat (h,w) span per d-plane
NV = DLOC * FL                # 1182 per scratch plane
NPASS = 3                     # one channel group per pass
CG = 16                       # channels per group
NJ = 27
NOUT = DLOC * HB * W          # 1152 true output voxels per row

# reduction engines: "pe" (identity matmul accumulate) or "dve" (pair tree)
RED_DEN = "pe"
RED_NUM = "pe"

F32 = mybir.dt.float32
BF16 = mybir.dt.bfloat16

_CACHE = {}


def _tap_geometry(g):
    """Per pass g: list over groups a of list of 3 qk/ev ops.

    Each op is (slot_base, koff, bstride) covering slots
    [slot_base, slot_base+3) with window offsets koff + i*bstride.
    Slot order per pass puts the rel axis outermost (slot = a*9 + o2*3 + i).
    """
    ops = []
    for a in range(3):
        row = []
        for o2 in range(3):
            if g == 0:    # a=wj, o2=dj, batch=hj
                dj, hj, wj, bs = o2, 0, a, WP
            elif g == 1:  # a=dj, o2=hj, batch=wj
                dj, hj, wj, bs = a, o2, 0, 1
            else:         # a=hj, o2=dj, batch=wj
                dj, hj, wj, bs = o2, a, 0, 1
            row.append((a * 9 + o2 * 3, dj * SL + hj * WP + wj, bs))
        ops.append(row)
    return ops


def _win_ap(flat, off, bstride):
    """[128, 3(batch), 3(d), 394] window view into a [128, 2640] tile."""
    base = flat[:, off:off + 1]
    return bass.AP(tensor=base.tensor, offset=base.offset,
                   ap=[base.ap[0], [bstride, 3], [SL, DLOC], [1, FL]])


def _q_ap(qt, rep):
    """[128, rep(broadcast), 3(d), 394] from a [128, NV] q tile."""
    base = qt[:, 0:1]
    return bass.AP(tensor=base.tensor, offset=base.offset,
                   ap=[base.ap[0], [0, rep], [FL, DLOC], [1, FL]])


def _scr_ap(scr, s0, n):
    """[128, n(slots), 3(d), 394] view of scratch slots [s0, s0+n)."""
    base = scr[:, s0, 0:1]
    return bass.AP(tensor=base.tensor, offset=base.offset,
                   ap=[base.ap[0], [NV, n], [FL, DLOC], [1, FL]])


def _dhw_ap(t, col0=0):
    """[128, 3(d), 6(h), 64(w)] true-voxel view of a [128, NV] plane tile."""
    base = t[:, col0:col0 + 1]
    return bass.AP(tensor=base.tensor, offset=base.offset,
                   ap=[base.ap[0], [FL, DLOC], [WP, HB], [1, W]])


def build_program():
    nc = bacc.Bacc("TRN2", target_bir_lowering=False, debug=False,
                   num_devices=NCORES)

    xs = nc.declare_dram_parameter("xs", [CIN, NS, DP, HBP, WP], BF16,
                                   isOutput=False)
    wq = nc.declare_dram_parameter("wq", [CIN, COUT], BF16, isOutput=False)
    wkv = nc.declare_dram_parameter("wkv", [CIN, 96], BF16, isOutput=False)
    relt = nc.declare_dram_parameter("relt", [NPASS, 128, 3], F32,
                                     isOutput=False)
    eye = nc.declare_dram_parameter("eye", [128, 128], BF16, isOutput=False)
    y = nc.declare_dram_parameter("y", [NPASS, 128, NOUT], BF16, isOutput=True)

    # psum column chunks for the PE reductions
    CH = [(0, 394), (394, 394), (788, 394)]

    with tile.TileContext(nc) as tc:
        with (
            tc.tile_pool(name="consts", bufs=1) as consts,
            tc.tile_pool(name="rows", bufs=1) as rows_pool,
            tc.tile_pool(name="attn", bufs=1) as attn,
            tc.tile_pool(name="kapool", bufs=2) as kapool,
        ):
            # ---- constants ----
            wq_sb = consts.tile([CIN, COUT], BF16, name="wq_sb")
            nc.sync.dma_start(out=wq_sb, in_=wq[:])
            wkv_sb = consts.tile([CIN, 96], BF16, name="wkv_sb")
            nc.sync.dma_start(out=wkv_sb, in_=wkv[:])
            rel_sb = consts.tile([128, NPASS, 3], F32, name="rel_sb")
            nc.sync.dma_start(out=rel_sb, in_=relt[:].rearrange("p r a -> r p a"))
            eye_sb = consts.tile([128, 128], BF16, name="eye_sb")
            nc.sync.dma_start(out=eye_sb, in_=eye[:])

            # ---- per-pass row tiles (all 3 passes resident) ----
            # kvf[g][:, 0] = k rows, kvf[g][:, 1] = v rows
            kvf = [rows_pool.tile([128, 2, BLK], BF16, name=f"kvf{g}")
                   for g in range(NPASS)]
            qt = [rows_pool.tile([128, NV], BF16, name=f"qt{g}")
                  for g in range(NPASS)]

            # ---- projection prologue, per s-block ----
            with (
                tc.tile_pool(name="psum", bufs=2, space="PSUM") as psum_pool,
                tc.tile_pool(name="stage", bufs=3) as stage,
                tc.tile_pool(name="xstage", bufs=8) as xstage,
            ):
              xsts = []
              for s in range(NS):
                xst = xstage.tile([CIN, BLK], BF16, tag="xst", name=f"xst{s}")
                dmae = nc.sync if s % 2 == 0 else nc.scalar
                dmae.dma_start(out=xst, in_=xs[:, s].rearrange(
                    "c d h w -> c (d h w)"))
                xsts.append(xst)
              for s in range(NS):
                xst = xsts[s]
                kvst = stage.tile([96, BLK], BF16, tag="kvst", name=f"kvst{s}")
                qst = stage.tile([COUT, DLOC, FL], BF16, tag="qst",
                                 name=f"qst{s}")
                for i in range(BLK // 440):
                    ps = psum_pool.tile([96, 440], F32, tag="pskv", name="pskv")
                    nc.tensor.matmul(ps, wkv_sb, xst[:, i * 440:(i + 1) * 440],
                                     start=True, stop=True)
                    if i % 2 == 0:
                        nc.vector.tensor_copy(
                            out=kvst[:, i * 440:(i + 1) * 440], in_=ps)
                    else:
                        nc.scalar.copy(kvst[:, i * 440:(i + 1) * 440], ps)
                # q on the block interior; one matmul per output d-plane
                for d in range(DLOC):
                    psq = psum_pool.tile([COUT, HB * W], F32, tag="psq",
                                         name="psq")
                    rb = xst[:, (d + 1) * SL + WP + 1:(d + 1) * SL + WP + 2]
                    rhs = bass.AP(tensor=rb.tensor, offset=rb.offset,
                                  ap=[rb.ap[0], [WP, HB], [1, W]])
                    nc.tensor.matmul(psq, wq_sb, rhs, start=True, stop=True)
                    qb = qst[:, d, 0:1]
                    qout = bass.AP(tensor=qb.tensor, offset=qb.offset,
                                   ap=[qb.ap[0], [WP, HB], [1, W]])
                    if d % 2 == 0:
                        nc.vector.tensor_copy(out=qout, in_=psq)
                    else:
                        nc.scalar.copy(qout, psq)
                # scatter stage rows into per-pass row tiles (SBUF->SBUF DMA).
                # wkv cols are (c, kv)-interleaved per group, so one DMA moves
                # k+v: src partitions (2c, 2c+1) -> dst row r0+c slots (0, 1).
                r0 = s * CG
                for g in range(NPASS):
                    dmae = nc.sync if (s + g) % 2 == 0 else nc.scalar
                    dmae.dma_start(out=kvf[g][r0:r0 + CG, 0],
                                   in_=kvst[g * CG:(g + 1) * CG])
                    dmae.dma_start(out=kvf[g][r0:r0 + CG, 1],
                                   in_=kvst[48 + g * CG:48 + (g + 1) * CG])
                    dmae.dma_start(
                        out=qt[g][r0:r0 + CG],
                        in_=qst[g * CG:(g + 1) * CG].rearrange(
                            "c d f -> c (d f)"))

            # ---- attention passes ----
            rpsum_ctx = tc.tile_pool(name="rpsum", bufs=1, space="PSUM")
            rpsum_pool = rpsum_ctx.__enter__()
            # scratch: per-group slot tiles so cross-pass deps stay fine-grained
            scrg = [attn.tile([128, 9, NV], BF16, name=f"scrg{a}")
                    for a in range(3)]
            nsum = attn.tile([128, NV], BF16, name="nsum")
            den32 = attn.tile([128, NV], F32, name="den32")
            rcp32 = attn.tile([128, NV], F32, name="rcp32")

            for g in range(NPASS):
                geo = _tap_geometry(g)
                kff, vff, qtg = kvf[g][:, 0], kvf[g][:, 1], qt[g]
                outt = attn.tile([128, NOUT], BF16, tag="outt", name="outt")
                psd = [rpsum_pool.tile([128, cw], F32, tag=f"psd{c0}",
                                       name="psd") for (c0, cw) in CH]
                psn = [rpsum_pool.tile([128, cw], F32, tag=f"psn{c0}",
                                       name="psn") for (c0, cw) in CH]

                # rel folded into k: ka = k + rel_a (per-partition scalar),
                # so e = exp(q*ka) needs no separate rel factor downstream.
                for a in range(3):
                    ka = kapool.tile([128, BLK], BF16, tag="ka", name="ka")
                    nc.gpsimd.tensor_scalar(
                        out=ka, in0=kff, scalar1=rel_sb[:, g, a:a + 1],
                        scalar2=None, op0=mybir.AluOpType.add)
                    for oi, (sb, koff, bs) in enumerate(geo[a]):
                        nc.vector.tensor_tensor(
                            out=_scr_ap(scrg[a], sb - a * 9, 3),
                            in0=_q_ap(qtg, 3),
                            in1=_win_ap(ka, koff, bs),
                            op=mybir.AluOpType.mult)
                    sub = scrg[a].rearrange("r j v -> r (j v)")
                    nc.scalar.activation(
                        out=sub, in_=sub,
                        func=mybir.ActivationFunctionType.Exp)
                    # accumulate denominator: all 27 planes into one psum/chunk
                    for ci, (c0, cw) in enumerate(CH):
                        for j in range(9):
                            nc.tensor.matmul(
                                psd[ci], eye_sb, scrg[a][:, j, c0:c0 + cw],
                                start=(a == 0 and j == 0),
                                stop=(a == 2 and j == 8))
                # den psum complete: evict early so next pass can reuse banks
                for ci, (c0, cw) in enumerate(CH):
                    nc.scalar.copy(den32[:, c0:c0 + cw], psd[ci])
                nc.vector.reciprocal_approx_fast(out=rcp32, in_=den32)
                # e <- e * v_win (in place), accumulate numerator
                for a in range(3):
                    for (sb, koff, bs) in geo[a]:
                        ap = _scr_ap(scrg[a], sb - a * 9, 3)
                        nc.vector.tensor_tensor(
                            out=ap, in0=ap, in1=_win_ap(vff, koff, bs),
                            op=mybir.AluOpType.mult)
                    for ci, (c0, cw) in enumerate(CH):
                        for j in range(9):
                            nc.tensor.matmul(
                                psn[ci], eye_sb, scrg[a][:, j, c0:c0 + cw],
                                start=(a == 0 and j == 0),
                                stop=(a == 2 and j == 8))
                for ci, (c0, cw) in enumerate(CH):
                    nc.scalar.copy(nsum[:, c0:c0 + cw], psn[ci])
                nc.gpsimd.tensor_tensor(
                    out=outt.rearrange("r (d h w) -> r d h w", d=DLOC, h=HB),
                    in0=_dhw_ap(nsum), in1=_dhw_ap(rcp32),
                    op=mybir.AluOpType.mult)
                nc.gpsimd.dma_start(out=y[g], in_=outt)
            rpsum_ctx.__exit__(None, None, None)
    nc.compile()
    return nc


def _host_prep(x, Wq, Wk, Wv, rel_h, rel_w, rel_d):
    import ml_dtypes
    tobf = lambda a: np.ascontiguousarray(a).astype(ml_dtypes.bfloat16)

    x = np.asarray(x, np.float32).reshape(CIN, D, H, W)
    xp = np.pad(x, ((0, 0), (1, 1), (1, 1), (1, 1)))  # (32, 26, 50, 66)
    wqT = np.ascontiguousarray(np.asarray(Wq, np.float32).T)
    wkvT = np.zeros((CIN, 96), np.float32)
    wkvT[:, 0:48] = np.asarray(Wk, np.float32).T
    wkvT[:, 48:96] = np.asarray(Wv, np.float32).T

    # relt[g, r, a]: pass g rows r=(s, c_sub); ch group g, rel axis value a
    rel_d2 = np.asarray(rel_d, np.float32).reshape(C3, K)  # ch 0-15, a=wj
    rel_h2 = np.asarray(rel_h, np.float32).reshape(C3, K)  # ch 16-31, a=dj
    rel_w2 = np.asarray(rel_w, np.float32).reshape(C3, K)  # ch 32-47, a=hj
    relt = np.zeros((NPASS, 128, 3), np.float32)
    csub = np.arange(128) % CG
    relt[0] = rel_d2[csub]
    relt[1] = rel_h2[csub]
    relt[2] = rel_w2[csub]

    eyem = np.eye(128, dtype=np.float32)

    in_maps = []
    for i in range(NCORES):
        slab = xp[:, 3 * i:3 * i + DP]  # (32, 5, 50, 66)
        xb = np.empty((CIN, NS, DP, HBP, WP), np.float32)
        for s in range(NS):
            xb[:, s] = slab[:, :, HB * s:HB * s + HBP, :]
        in_maps.append({
            "xs": tobf(xb), "wq": tobf(wqT), "wkv": tobf(wkvT),
            "relt": relt, "eye": tobf(eyem),
        })
    return in_maps


def kernel(x, Wq, Wk, Wv, rel_h, rel_w, rel_d, trace=False):
    in_maps = _host_prep(x, Wq, Wk, Wv, rel_h, rel_w, rel_d)
    if "nc" not in _CACHE:
        _CACHE["nc"] = build_program()
    res = run_bass_kernel_spmd(
        _CACHE["nc"], in_maps, core_ids=list(range(NCORES)), trace=trace)
    # y per core: (NPASS, 128, NOUT); row r=(s, c_sub) of pass g
    out = np.zeros((COUT, D, H, W), np.float32)
    for i in range(NCORES):
        yv = np.asarray(res.results[i]["y"]).astype(np.float32)
        yv = yv.reshape(NPASS, NS, CG, DLOC, HB, W)
        for g in range(NPASS):
            for s in range(NS):
                out[g * CG:(g + 1) * CG, 3 * i:3 * i + DLOC,
                    HB * s:HB * s + HB, :] = yv[g, s]
    if trace:
        _CACHE["last"] = res
    return out.reshape(1, COUT, D, H, W)


# revision 38
# speedup vs baseline: 2.4990x; 2.4990x over previous
"""Trainium2 Bass kernel for 3D windowed (3x3x3) per-channel softmax attention.

Problem (hardcoded): x (1,32,24,48,64) f32; Wq/Wk/Wv (48,32); rel_* (16,...,3).
  q = Wq@x ; kf/vf = Wk/Wv @ pad(x) ; per (c,voxel): softmax over the 27
  window taps of q*(k_win+rel), then weighted sum of v_win.

Strategy (v2):
  - Shard D=24 across 8 cores (3 output d-slices each + 1-voxel halo,
    zero-padded on host). SPMD, no collectives.
  - Rows r = (s, c_sub): 8 H-blocks x 16 channels = 128 partitions/pass,
    3 passes = one channel GROUP of 16 per pass. Within a pass every
    channel shares the same rel axis (ch 0-15: rel_d varies over wj,
    16-31: rel_h over dj, 32-47: rel_w over hj), so
      exp(q*(k+rel_j)) = exp(q*k_j) * F_a,   F_a = exp(q*rel_a)
    factors with only 3 F planes per pass.
  - qk logits via plain tensor_tensor (2x DVE mode; the baseline's
    scalar_tensor_tensor runs at 1x), batched 3 taps per op via window APs.
  - exp on ACT (in-place over the logit planes).
  - Per-axis sums S_a = sum_{j in a} e_j and T_a = sum_{j in a} e_j*v_j
    either on PE (identity-matmul PSUM accumulation, Pool evicts) or on
    DVE (pairwise trees) - configurable via RED_DEN/RED_NUM.
  - den = sum_a F_a*S_a, num = sum_a F_a*T_a, out = num * recip(den).
  - No DRAM bounce: projections go PSUM -> SBUF stage -> SBUF-SBUF DMA
    gather into per-pass row tiles.
"""

import sys

sys.path.insert(0, "/opt/trn_rl_repo")

import numpy as np

import concourse.bass as bass
import concourse.bacc as bacc
import concourse.mybir as mybir
import concourse.tile as tile
from concourse.bass_utils import run_bass_kernel_spmd

# ---- problem constants (hardcoded per contract) ----
B, CIN, D, H, W = 1, 32, 24, 48, 64
COUT, K, C3 = 48, 3, 16
NCORES = 8
DLOC = D // NCORES            # 3 output d-slices per core
DP = DLOC + 2                 # 5 padded d-planes per core
NS = 8                        # H-blocks per core
HB = H // NS                  # 6 output rows per block
HBP = HB + 2                  # 8 padded rows per block
WP = W + 2                    # 66
BLK = DP * HBP * WP           # 2640 padded voxels per block
SL = HBP * WP                 # 528: one padded d-plane
FL = (HB - 1) * WP + W        # 394-elem flat (h,w) span per d-plane
NV = DLOC * FL                # 1182 per scratch plane
NPASS = 3                     # one channel group per pass
CG = 16                       # channels per group
NJ = 27
NOUT = DLOC * HB * W          # 1152 true output voxels per row

# reduction engines: "pe" (identity matmul accumulate) or "dve" (pair tree)
RED_DEN = "pe"
RED_NUM = "pe"

F32 = mybir.dt.float32
BF16 = mybir.dt.bfloat16

_CACHE = {}


def _tap_geometry(g):
    """Per pass g: list over groups a of list of 3 qk/ev ops.

    Each op is (slot_base, koff, bstride) covering slots
    [slot_base, slot_base+3) with window offsets koff + i*bstride.
    Slot order per pass puts the rel axis outermost (slot = a*9 + o2*3 + i).
    """
    ops = []
    for a in range(3):
        row = []
        for o2 in range(3):
            if g == 0:    # a=wj, o2=dj, batch=hj
                dj, hj, wj, bs = o2, 0, a, WP
            elif g == 1:  # a=dj, o2=hj, batch=wj
                dj, hj, wj, bs = a, o2, 0, 1
            else:         # a=hj, o2=dj, batch=wj
                dj, hj, wj, bs = o2, a, 0, 1
            row.append((a * 9 + o2 * 3, dj * SL + hj * WP + wj, bs))
        ops.append(row)
    return ops


def _win_ap(flat, off, bstride):
    """[128, 3(batch), 3(d), 394] window view into a [128, 2640] tile."""
    base = flat[:, off:off + 1]
    return bass.AP(tensor=base.tensor, offset=base.offset,
                   ap=[base.ap[0], [bstride, 3], [SL, DLOC], [1, FL]])


def _q_ap(qt, rep):
    """[128, rep(broadcast), 3(d), 394] from a [128, NV] q tile."""
    base = qt[:, 0:1]
    return bass.AP(tensor=base.tensor, offset=base.offset,
                   ap=[base.ap[0], [0, rep], [FL, DLOC], [1, FL]])


def _scr_ap(scr, s0, n):
    """[128, n(slots), 3(d), 394] view of scratch slots [s0, s0+n)."""
    base = scr[:, s0, 0:1]
    return bass.AP(tensor=base.tensor, offset=base.offset,
                   ap=[base.ap[0], [NV, n], [FL, DLOC], [1, FL]])


def _dhw_ap(t, col0=0):
    """[128, 3(d), 6(h), 64(w)] true-voxel view of a [128, NV] plane tile."""
    base = t[:, col0:col0 + 1]
    return bass.AP(tensor=base.tensor, offset=base.offset,
                   ap=[base.ap[0], [FL, DLOC], [WP, HB], [1, W]])


def build_program():
    nc = bacc.Bacc("TRN2", target_bir_lowering=False, debug=False,
                   num_devices=NCORES)

    xs = nc.declare_dram_parameter("xs", [CIN, NS, DP, HBP, WP], BF16,
                                   isOutput=False)
    wq = nc.declare_dram_parameter("wq", [CIN, COUT], BF16, isOutput=False)
    wkv = nc.declare_dram_parameter("wkv", [CIN, 96], BF16, isOutput=False)
    relt = nc.declare_dram_parameter("relt", [NPASS, 128, 3], F32,
                                     isOutput=False)
    eye = nc.declare_dram_parameter("eye", [128, 128], BF16, isOutput=False)
    y = nc.declare_dram_parameter("y", [NPASS, 128, NOUT], BF16, isOutput=True)

    # psum column chunks for the PE reductions
    CH = [(0, 394), (394, 394), (788, 394)]

    with tile.TileContext(nc) as tc:
        with (
            tc.tile_pool(name="consts", bufs=1) as consts,
            tc.tile_pool(name="rows", bufs=1) as rows_pool,
            tc.tile_pool(name="attn", bufs=1) as attn,
            tc.tile_pool(name="kapool", bufs=2) as kapool,
        ):
            # ---- constants ----
            wq_sb = consts.tile([CIN, COUT], BF16, name="wq_sb")
            nc.sync.dma_start(out=wq_sb, in_=wq[:])
            wkv_sb = consts.tile([CIN, 96], BF16, name="wkv_sb")
            nc.sync.dma_start(out=wkv_sb, in_=wkv[:])
            rel_sb = consts.tile([128, NPASS, 3], F32, name="rel_sb")
            nc.sync.dma_start(out=rel_sb, in_=relt[:].rearrange("p r a -> r p a"))
            eye_sb = consts.tile([128, 128], BF16, name="eye_sb")
            nc.sync.dma_start(out=eye_sb, in_=eye[:])

            # ---- per-pass row tiles (all 3 passes resident) ----
            # kvf[g][:, 0] = k rows, kvf[g][:, 1] = v rows
            kvf = [rows_pool.tile([128, 2, BLK], BF16, name=f"kvf{g}")
                   for g in range(NPASS)]
            qt = [rows_pool.tile([128, NV], BF16, name=f"qt{g}")
                  for g in range(NPASS)]

            # ---- projection prologue, per s-block ----
            with (
                tc.tile_pool(name="psum", bufs=2, space="PSUM") as psum_pool,
                tc.tile_pool(name="stage", bufs=8) as stage,
                tc.tile_pool(name="qstage", bufs=2) as qstage,
                tc.tile_pool(name="xstage", bufs=2) as xstage,
            ):
              xsts, kvsts, qsts = [], [], []
              for s in range(NS):
                xst = xstage.tile([CIN, BLK], BF16, tag="xst", name=f"xst{s}")
                dmae = nc.sync if s % 2 == 0 else nc.scalar
                dmae.dma_start(out=xst, in_=xs[:, s].rearrange(
                    "c d h w -> c (d h w)"))
                xsts.append(xst)
              for s in range(NS):
                xst = xsts[s]
                kvst = stage.tile([96, BLK], BF16, tag="kvst", name=f"kvst{s}")
                qst = qstage.tile([COUT, DLOC, FL], BF16, tag="qst",
                                  name=f"qst{s}")
                for i in range(BLK // 440):
                    ps = psum_pool.tile([96, 440], F32, tag="pskv", name="pskv")
                    nc.tensor.matmul(ps, wkv_sb, xst[:, i * 440:(i + 1) * 440],
                                     start=True, stop=True)
                    if i % 2 == 0:
                        nc.vector.tensor_copy(
                            out=kvst[:, i * 440:(i + 1) * 440], in_=ps)
                    else:
                        nc.scalar.copy(kvst[:, i * 440:(i + 1) * 440], ps)
                # q on the block interior; one matmul per output d-plane
                for d in range(DLOC):
                    psq = psum_pool.tile([COUT, HB * W], F32, tag="psq",
                                         name="psq")
                    rb = xst[:, (d + 1) * SL + WP + 1:(d + 1) * SL + WP + 2]
                    rhs = bass.AP(tensor=rb.tensor, offset=rb.offset,
                                  ap=[rb.ap[0], [WP, HB], [1, W]])
                    nc.tensor.matmul(psq, wq_sb, rhs, start=True, stop=True)
                    qb = qst[:, d, 0:1]
                    qout = bass.AP(tensor=qb.tensor, offset=qb.offset,
                                   ap=[qb.ap[0], [WP, HB], [1, W]])
                    if d % 2 == 0:
                        nc.vector.tensor_copy(out=qout, in_=psq)
                    else:
                        nc.scalar.copy(qout, psq)
                kvsts.append(kvst)
                # q gathers inline (small); kv gathers deferred below
                r0 = s * CG
                for g in range(NPASS):
                    dmae = nc.sync if (s + g) % 2 == 0 else nc.scalar
                    dmae.dma_start(
                        out=qt[g][r0:r0 + CG],
                        in_=qst[g * CG:(g + 1) * CG].rearrange(
                            "c d f -> c (d f)"))
              # kv gathers: pass-0 first so attention starts as early as
              # possible; later passes' transfers overlap pass-0 compute
              for g in range(NPASS):
                for s in range(NS):
                    r0 = s * CG
                    kvst = kvsts[s]
                    dmae = nc.sync if (s + g) % 2 == 0 else nc.scalar
                    dmae.dma_start(out=kvf[g][r0:r0 + CG, 0],
                                   in_=kvst[g * CG:(g + 1) * CG])
                    dmae.dma_start(out=kvf[g][r0:r0 + CG, 1],
                                   in_=kvst[48 + g * CG:48 + (g + 1) * CG])

            # ---- attention passes ----
            rpsum_ctx = tc.tile_pool(name="rpsum", bufs=1, space="PSUM")
            rpsum_pool = rpsum_ctx.__enter__()
            # scratch: per-group slot tiles so cross-pass deps stay fine-grained
            scrg = [attn.tile([128, 9, NV], BF16, name=f"scrg{a}")
                    for a in range(3)]
            nsum = attn.tile([128, NV], BF16, name="nsum")
            den32 = attn.tile([128, NV], F32, name="den32")
            rcp32 = attn.tile([128, NV], F32, name="rcp32")

            for g in range(NPASS):
                geo = _tap_geometry(g)
                kff, vff, qtg = kvf[g][:, 0], kvf[g][:, 1], qt[g]
                outt = attn.tile([128, NOUT], BF16, tag="outt", name="outt")
                psd = [rpsum_pool.tile([128, cw], F32, tag=f"psd{c0}",
                                       name="psd") for (c0, cw) in CH]
                psn = [rpsum_pool.tile([128, cw], F32, tag=f"psn{c0}",
                                       name="psn") for (c0, cw) in CH]

                # rel folded into k: ka = k + rel_a (per-partition scalar),
                # so e = exp(q*ka) needs no separate rel factor downstream.
                for a in range(3):
                    ka = kapool.tile([128, BLK], BF16, tag="ka", name="ka")
                    nc.vector.tensor_scalar(
                        out=ka, in0=kff, scalar1=rel_sb[:, g, a:a + 1],
                        scalar2=None, op0=mybir.AluOpType.add)
                    for oi, (sb, koff, bs) in enumerate(geo[a]):
                        nc.vector.tensor_tensor(
                            out=_scr_ap(scrg[a], sb - a * 9, 3),
                            in0=_q_ap(qtg, 3),
                            in1=_win_ap(ka, koff, bs),
                            op=mybir.AluOpType.mult)
                    sub = scrg[a].rearrange("r j v -> r (j v)")
                    nc.scalar.activation(
                        out=sub, in_=sub,
                        func=mybir.ActivationFunctionType.Exp)
                    # accumulate denominator: all 27 planes into one psum/chunk
                    for ci, (c0, cw) in enumerate(CH):
                        for j in range(9):
                            nc.tensor.matmul(
                                psd[ci], eye_sb, scrg[a][:, j, c0:c0 + cw],
                                start=(a == 0 and j == 0),
                                stop=(a == 2 and j == 8))
                # den psum complete: evict early so next pass can reuse banks
                for ci, (c0, cw) in enumerate(CH):
                    nc.scalar.copy(den32[:, c0:c0 + cw], psd[ci])
                nc.vector.reciprocal_approx_fast(out=rcp32, in_=den32)
                # e <- e * v_win (in place), accumulate numerator
                for a in range(3):
                    for (sb, koff, bs) in geo[a]:
                        ap = _scr_ap(scrg[a], sb - a * 9, 3)
                        nc.vector.tensor_tensor(
                            out=ap, in0=ap, in1=_win_ap(vff, koff, bs),
                            op=mybir.AluOpType.mult)
                    for ci, (c0, cw) in enumerate(CH):
                        for j in range(9):
                            nc.tensor.matmul(
                                psn[ci], eye_sb, scrg[a][:, j, c0:c0 + cw],
                                start=(a == 0 and j == 0),
                                stop=(a == 2 and j == 8))
                for ci, (c0, cw) in enumerate(CH):
                    nc.scalar.copy(nsum[:, c0:c0 + cw], psn[ci])
                nc.vector.tensor_tensor(
                    out=outt.rearrange("r (d h w) -> r d h w", d=DLOC, h=HB),
                    in0=_dhw_ap(nsum), in1=_dhw_ap(rcp32),
                    op=mybir.AluOpType.mult)
                nc.gpsimd.dma_start(out=y[g], in_=outt)
            rpsum_ctx.__exit__(None, None, None)
    nc.compile()
    return nc


def _host_prep(x, Wq, Wk, Wv, rel_h, rel_w, rel_d):
    import ml_dtypes
    tobf = lambda a: np.ascontiguousarray(a).astype(ml_dtypes.bfloat16)

    x = np.asarray(x, np.float32).reshape(CIN, D, H, W)
    xp = np.pad(x, ((0, 0), (1, 1), (1, 1), (1, 1)))  # (32, 26, 50, 66)
    wqT = np.ascontiguousarray(np.asarray(Wq, np.float32).T)
    wkvT = np.zeros((CIN, 96), np.float32)
    wkvT[:, 0:48] = np.asarray(Wk, np.float32).T
    wkvT[:, 48:96] = np.asarray(Wv, np.float32).T

    # relt[g, r, a]: pass g rows r=(s, c_sub); ch group g, rel axis value a
    rel_d2 = np.asarray(rel_d, np.float32).reshape(C3, K)  # ch 0-15, a=wj
    rel_h2 = np.asarray(rel_h, np.float32).reshape(C3, K)  # ch 16-31, a=dj
    rel_w2 = np.asarray(rel_w, np.float32).reshape(C3, K)  # ch 32-47, a=hj
    relt = np.zeros((NPASS, 128, 3), np.float32)
    csub = np.arange(128) % CG
    relt[0] = rel_d2[csub]
    relt[1] = rel_h2[csub]
    relt[2] = rel_w2[csub]

    eyem = np.eye(128, dtype=np.float32)

    in_maps = []
    for i in range(NCORES):
        slab = xp[:, 3 * i:3 * i + DP]  # (32, 5, 50, 66)
        xb = np.empty((CIN, NS, DP, HBP, WP), np.float32)
        for s in range(NS):
            xb[:, s] = slab[:, :, HB * s:HB * s + HBP, :]
        in_maps.append({
            "xs": tobf(xb), "wq": tobf(wqT), "wkv": tobf(wkvT),
            "relt": relt, "eye": tobf(eyem),
        })
    return in_maps


def kernel(x, Wq, Wk, Wv, rel_h, rel_w, rel_d, trace=False):
    in_maps = _host_prep(x, Wq, Wk, Wv, rel_h, rel_w, rel_d)
    if "nc" not in _CACHE:
        _CACHE["nc"] = build_program()
    res = run_bass_kernel_spmd(
        _CACHE["nc"], in_maps, core_ids=list(range(NCORES)), trace=trace)
    # y per core: (NPASS, 128, NOUT); row r=(s, c_sub) of pass g
    out = np.zeros((COUT, D, H, W), np.float32)
    for i in range(NCORES):
        yv = np.asarray(res.results[i]["y"]).astype(np.float32)
        yv = yv.reshape(NPASS, NS, CG, DLOC, HB, W)
        for g in range(NPASS):
            for s in range(NS):
                out[g * CG:(g + 1) * CG, 3 * i:3 * i + DLOC,
                    HB * s:HB * s + HB, :] = yv[g, s]
    if trace:
        _CACHE["last"] = res
    return out.reshape(1, COUT, D, H, W)


# revision 43
# speedup vs baseline: 2.5718x; 1.0291x over previous
"""Trainium2 Bass kernel for 3D windowed (3x3x3) per-channel softmax attention.

Problem (hardcoded): x (1,32,24,48,64) f32; Wq/Wk/Wv (48,32); rel_* (16,...,3).
  q = Wq@x ; kf/vf = Wk/Wv @ pad(x) ; per (c,voxel): softmax over the 27
  window taps of q*(k_win+rel), then weighted sum of v_win.

Strategy (v2):
  - Shard D=24 across 8 cores (3 output d-slices each + 1-voxel halo,
    zero-padded on host). SPMD, no collectives.
  - Rows r = (s, c_sub): 8 H-blocks x 16 channels = 128 partitions/pass,
    3 passes = one channel GROUP of 16 per pass. Within a pass every
    channel shares the same rel axis (ch 0-15: rel_d varies over wj,
    16-31: rel_h over dj, 32-47: rel_w over hj), so
      exp(q*(k+rel_j)) = exp(q*k_j) * F_a,   F_a = exp(q*rel_a)
    factors with only 3 F planes per pass.
  - qk logits via plain tensor_tensor (2x DVE mode; the baseline's
    scalar_tensor_tensor runs at 1x), batched 3 taps per op via window APs.
  - exp on ACT (in-place over the logit planes).
  - Per-axis sums S_a = sum_{j in a} e_j and T_a = sum_{j in a} e_j*v_j
    either on PE (identity-matmul PSUM accumulation, Pool evicts) or on
    DVE (pairwise trees) - configurable via RED_DEN/RED_NUM.
  - den = sum_a F_a*S_a, num = sum_a F_a*T_a, out = num * recip(den).
  - No DRAM bounce: projections go PSUM -> SBUF stage -> SBUF-SBUF DMA
    gather into per-pass row tiles.
"""

import sys

sys.path.insert(0, "/opt/trn_rl_repo")

import numpy as np

import concourse.bass as bass
import concourse.bacc as bacc
import concourse.mybir as mybir
import concourse.tile as tile
from concourse.bass_utils import run_bass_kernel_spmd

# ---- problem constants (hardcoded per contract) ----
B, CIN, D, H, W = 1, 32, 24, 48, 64
COUT, K, C3 = 48, 3, 16
NCORES = 8
DLOC = D // NCORES            # 3 output d-slices per core
DP = DLOC + 2                 # 5 padded d-planes per core
NS = 8                        # H-blocks per core
HB = H // NS                  # 6 output rows per block
HBP = HB + 2                  # 8 padded rows per block
WP = W + 2                    # 66
BLK = DP * HBP * WP           # 2640 padded voxels per block
SL = HBP * WP                 # 528: one padded d-plane
FL = (HB - 1) * WP + W        # 394-elem flat (h,w) span per d-plane
NV = DLOC * FL                # 1182 per scratch plane
NPASS = 3                     # one channel group per pass
CG = 16                       # channels per group
NJ = 27
NOUT = DLOC * HB * W          # 1152 true output voxels per row

# reduction engines: "pe" (identity matmul accumulate) or "dve" (pair tree)
RED_DEN = "pe"
RED_NUM = "pe"

F32 = mybir.dt.float32
BF16 = mybir.dt.bfloat16

_CACHE = {}


def _tap_geometry(g):
    """Per pass g: list over groups a of list of 3 qk/ev ops.

    Each op is (slot_base, koff, bstride) covering slots
    [slot_base, slot_base+3) with window offsets koff + i*bstride.
    Slot order per pass puts the rel axis outermost (slot = a*9 + o2*3 + i).
    """
    ops = []
    for a in range(3):
        row = []
        for o2 in range(3):
            if g == 0:    # a=wj, o2=dj, batch=hj
                dj, hj, wj, bs = o2, 0, a, WP
            elif g == 1:  # a=dj, o2=hj, batch=wj
                dj, hj, wj, bs = a, o2, 0, 1
            else:         # a=hj, o2=dj, batch=wj
                dj, hj, wj, bs = o2, a, 0, 1
            row.append((a * 9 + o2 * 3, dj * SL + hj * WP + wj, bs))
        ops.append(row)
    return ops


def _win_ap(flat, off, bstride):
    """[128, 3(batch), 3(d), 394] window view into a [128, 2640] tile."""
    base = flat[:, off:off + 1]
    return bass.AP(tensor=base.tensor, offset=base.offset,
                   ap=[base.ap[0], [bstride, 3], [SL, DLOC], [1, FL]])


def _q_ap(qt, rep):
    """[128, rep(broadcast), 3(d), 394] from a [128, NV] q tile."""
    base = qt[:, 0:1]
    return bass.AP(tensor=base.tensor, offset=base.offset,
                   ap=[base.ap[0], [0, rep], [FL, DLOC], [1, FL]])


def _scr_ap(scr, s0, n):
    """[128, n(slots), 3(d), 394] view of scratch slots [s0, s0+n)."""
    base = scr[:, s0, 0:1]
    return bass.AP(tensor=base.tensor, offset=base.offset,
                   ap=[base.ap[0], [NV, n], [FL, DLOC], [1, FL]])


def _dhw_ap(t, col0=0):
    """[128, 3(d), 6(h), 64(w)] true-voxel view of a [128, NV] plane tile."""
    base = t[:, col0:col0 + 1]
    return bass.AP(tensor=base.tensor, offset=base.offset,
                   ap=[base.ap[0], [FL, DLOC], [WP, HB], [1, W]])


def build_program():
    nc = bacc.Bacc("TRN2", target_bir_lowering=False, debug=False,
                   num_devices=NCORES)

    xs = nc.declare_dram_parameter("xs", [CIN, DP, H + 2, WP], BF16,
                                   isOutput=False)
    wq = nc.declare_dram_parameter("wq", [CIN, COUT], BF16, isOutput=False)
    wkv = nc.declare_dram_parameter("wkv", [CIN, 96], BF16, isOutput=False)
    relt = nc.declare_dram_parameter("relt", [NPASS, 128, 3], F32,
                                     isOutput=False)
    eye = nc.declare_dram_parameter("eye", [128, 128], BF16, isOutput=False)
    y = nc.declare_dram_parameter("y", [NPASS, 128, NOUT], BF16, isOutput=True)

    # psum column chunks for the PE reductions
    CH = [(0, 394), (394, 394), (788, 394)]

    with tile.TileContext(nc) as tc:
        with (
            tc.tile_pool(name="consts", bufs=1) as consts,
            tc.tile_pool(name="rows", bufs=1) as rows_pool,
            tc.tile_pool(name="attn", bufs=1) as attn,
            tc.tile_pool(name="kapool", bufs=1) as kapool,
        ):
            # ---- constants ----
            wq_sb = consts.tile([CIN, COUT], BF16, name="wq_sb")
            nc.sync.dma_start(out=wq_sb, in_=wq[:])
            wkv_sb = consts.tile([CIN, 96], BF16, name="wkv_sb")
            nc.sync.dma_start(out=wkv_sb, in_=wkv[:])
            rel_sb = consts.tile([128, NPASS, 3], F32, name="rel_sb")
            nc.sync.dma_start(out=rel_sb, in_=relt[:].rearrange("p r a -> r p a"))
            eye_sb = consts.tile([128, 128], BF16, name="eye_sb")
            nc.sync.dma_start(out=eye_sb, in_=eye[:])

            # ---- per-pass row tiles (all 3 passes resident) ----
            # kvf[g][:, 0] = k rows, kvf[g][:, 1] = v rows
            kvf = [rows_pool.tile([128, 2, BLK], BF16, name=f"kvf{g}")
                   for g in range(NPASS)]
            qt = [rows_pool.tile([128, NV], BF16, name=f"qt{g}")
                  for g in range(NPASS)]

            # ---- projection prologue: whole padded slab in one burst ----
            HP = H + 2                    # 50 padded h-rows
            SLAB = DP * HP * WP           # 16500
            QFL = (H - 1) * WP + W        # 3166: full-H flat span per d-plane
            with (
                tc.tile_pool(name="psum", bufs=4, space="PSUM") as psum_pool,
                tc.tile_pool(name="stage", bufs=1) as stage,
            ):
              xsl = stage.tile([CIN, SLAB], BF16, name="xsl")
              nc.sync.dma_start(out=xsl, in_=xs[:].rearrange(
                  "c d h w -> c (d h w)"))
              kvstF = stage.tile([96, SLAB], BF16, name="kvstF")
              qstF = stage.tile([COUT, DLOC, QFL], BF16, name="qstF")
              nchunk = (SLAB + 439) // 440
              for i in range(nchunk):
                    c0 = i * 440
                    cw = min(440, SLAB - c0)
                    ps = psum_pool.tile([96, 440], F32, tag="pskv", name="pskv")
                    nc.tensor.matmul(ps[:, 0:cw], wkv_sb, xsl[:, c0:c0 + cw],
                                     start=True, stop=True)
                    if i % 2 == 0:
                        nc.vector.tensor_copy(
                            out=kvstF[:, c0:c0 + cw], in_=ps[:, 0:cw])
                    else:
                        nc.scalar.copy(kvstF[:, c0:c0 + cw], ps[:, 0:cw])
              # q on the interior; chunks of 8 h-rows (512 cols)
              for d in range(DLOC):
                for hb in range(H // 8):
                    psq = psum_pool.tile([COUT, 512], F32, tag="psq",
                                         name="psq")
                    off = (d + 1) * HP * WP + (1 + 8 * hb) * WP + 1
                    rb = xsl[:, off:off + 1]
                    rhs = bass.AP(tensor=rb.tensor, offset=rb.offset,
                                  ap=[rb.ap[0], [WP, 8], [1, W]])
                    nc.tensor.matmul(psq, wq_sb, rhs, start=True, stop=True)
                    qb = qstF[:, d, 8 * hb * WP:8 * hb * WP + 1]
                    qout = bass.AP(tensor=qb.tensor, offset=qb.offset,
                                   ap=[qb.ap[0], [WP, 8], [1, W]])
                    if (d + hb) % 2 == 0:
                        nc.vector.tensor_copy(out=qout, in_=psq)
                    else:
                        nc.scalar.copy(qout, psq)
              # gathers: slice H-block windows; pass-0 first
              kvstV = kvstF.rearrange("c (d h w) -> c d h w", d=DP, h=HP)
              for g in range(NPASS):
                for s in range(NS):
                    r0 = s * CG
                    dmae = nc.sync if (s + g) % 2 == 0 else nc.scalar
                    dmae.dma_start(
                        out=kvf[g][r0:r0 + CG, 0],
                        in_=kvstV[g * CG:(g + 1) * CG, :, HB * s:HB * s + HBP])
                    dmae.dma_start(
                        out=kvf[g][r0:r0 + CG, 1],
                        in_=kvstV[48 + g * CG:48 + (g + 1) * CG, :,
                                  HB * s:HB * s + HBP])
                    dmae.dma_start(
                        out=qt[g][r0:r0 + CG],
                        in_=qstF[g * CG:(g + 1) * CG, :,
                                 HB * s * WP:HB * s * WP + FL])

            # ---- attention passes ----
            rpsum_ctx = tc.tile_pool(name="rpsum", bufs=1, space="PSUM")
            rpsum_pool = rpsum_ctx.__enter__()
            # scratch: per-group slot tiles so cross-pass deps stay fine-grained
            scrg = [attn.tile([128, 9, NV], BF16, name=f"scrg{a}")
                    for a in range(3)]
            nsum = attn.tile([128, NV], BF16, name="nsum")
            den32 = attn.tile([128, NV], F32, name="den32")
            rcp32 = attn.tile([128, NV], F32, name="rcp32")

            for g in range(NPASS):
                geo = _tap_geometry(g)
                kff, vff, qtg = kvf[g][:, 0], kvf[g][:, 1], qt[g]
                outt = attn.tile([128, NOUT], BF16, tag="outt", name="outt")
                psd = [rpsum_pool.tile([128, cw], F32, tag=f"psd{c0}",
                                       name="psd") for (c0, cw) in CH]
                psn = [rpsum_pool.tile([128, cw], F32, tag=f"psn{c0}",
                                       name="psn") for (c0, cw) in CH]

                # rel folded into k: ka = k + rel_a (per-partition scalar),
                # so e = exp(q*ka) needs no separate rel factor downstream.
                for a in range(3):
                    ka = kapool.tile([128, BLK], BF16, tag="ka", name="ka")
                    nc.vector.tensor_scalar(
                        out=ka, in0=kff, scalar1=rel_sb[:, g, a:a + 1],
                        scalar2=None, op0=mybir.AluOpType.add)
                    for oi, (sb, koff, bs) in enumerate(geo[a]):
                        nc.vector.tensor_tensor(
                            out=_scr_ap(scrg[a], sb - a * 9, 3),
                            in0=_q_ap(qtg, 3),
                            in1=_win_ap(ka, koff, bs),
                            op=mybir.AluOpType.mult)
                    sub = scrg[a].rearrange("r j v -> r (j v)")
                    nc.scalar.activation(
                        out=sub, in_=sub,
                        func=mybir.ActivationFunctionType.Exp)
                    # accumulate denominator: all 27 planes into one psum/chunk
                    for ci, (c0, cw) in enumerate(CH):
                        for j in range(9):
                            nc.tensor.matmul(
                                psd[ci], eye_sb, scrg[a][:, j, c0:c0 + cw],
                                start=(a == 0 and j == 0),
                                stop=(a == 2 and j == 8))
                # den psum complete: evict early so next pass can reuse banks
                for ci, (c0, cw) in enumerate(CH):
                    nc.scalar.copy(den32[:, c0:c0 + cw], psd[ci])
                nc.vector.reciprocal_approx_fast(out=rcp32, in_=den32)
                # e <- e * v_win (in place), accumulate numerator
                for a in range(3):
                    for (sb, koff, bs) in geo[a]:
                        ap = _scr_ap(scrg[a], sb - a * 9, 3)
                        nc.vector.tensor_tensor(
                            out=ap, in0=ap, in1=_win_ap(vff, koff, bs),
                            op=mybir.AluOpType.mult)
                    for ci, (c0, cw) in enumerate(CH):
                        for j in range(9):
                            nc.tensor.matmul(
                                psn[ci], eye_sb, scrg[a][:, j, c0:c0 + cw],
                                start=(a == 0 and j == 0),
                                stop=(a == 2 and j == 8))
                for ci, (c0, cw) in enumerate(CH):
                    nc.scalar.copy(nsum[:, c0:c0 + cw], psn[ci])
                nc.vector.tensor_tensor(
                    out=outt.rearrange("r (d h w) -> r d h w", d=DLOC, h=HB),
                    in0=_dhw_ap(nsum), in1=_dhw_ap(rcp32),
                    op=mybir.AluOpType.mult)
                nc.gpsimd.dma_start(out=y[g], in_=outt)
            rpsum_ctx.__exit__(None, None, None)
    nc.compile()
    return nc


def _host_prep(x, Wq, Wk, Wv, rel_h, rel_w, rel_d):
    import ml_dtypes
    tobf = lambda a: np.ascontiguousarray(a).astype(ml_dtypes.bfloat16)

    x = np.asarray(x, np.float32).reshape(CIN, D, H, W)
    xp = np.pad(x, ((0, 0), (1, 1), (1, 1), (1, 1)))  # (32, 26, 50, 66)
    wqT = np.ascontiguousarray(np.asarray(Wq, np.float32).T)
    wkvT = np.zeros((CIN, 96), np.float32)
    wkvT[:, 0:48] = np.asarray(Wk, np.float32).T
    wkvT[:, 48:96] = np.asarray(Wv, np.float32).T

    # relt[g, r, a]: pass g rows r=(s, c_sub); ch group g, rel axis value a
    rel_d2 = np.asarray(rel_d, np.float32).reshape(C3, K)  # ch 0-15, a=wj
    rel_h2 = np.asarray(rel_h, np.float32).reshape(C3, K)  # ch 16-31, a=dj
    rel_w2 = np.asarray(rel_w, np.float32).reshape(C3, K)  # ch 32-47, a=hj
    relt = np.zeros((NPASS, 128, 3), np.float32)
    csub = np.arange(128) % CG
    relt[0] = rel_d2[csub]
    relt[1] = rel_h2[csub]
    relt[2] = rel_w2[csub]

    eyem = np.eye(128, dtype=np.float32)

    in_maps = []
    for i in range(NCORES):
        slab = xp[:, 3 * i:3 * i + DP]  # (32, 5, 50, 66)
        in_maps.append({
            "xs": tobf(slab), "wq": tobf(wqT), "wkv": tobf(wkvT),
            "relt": relt, "eye": tobf(eyem),
        })
    return in_maps


def kernel(x, Wq, Wk, Wv, rel_h, rel_w, rel_d, trace=False):
    in_maps = _host_prep(x, Wq, Wk, Wv, rel_h, rel_w, rel_d)
    if "nc" not in _CACHE:
        _CACHE["nc"] = build_program()
    res = run_bass_kernel_spmd(
        _CACHE["nc"], in_maps, core_ids=list(range(NCORES)), trace=trace)
    # y per core: (NPASS, 128, NOUT); row r=(s, c_sub) of pass g
    out = np.zeros((COUT, D, H, W), np.float32)
    for i in range(NCORES):
        yv = np.asarray(res.results[i]["y"]).astype(np.float32)
        yv = yv.reshape(NPASS, NS, CG, DLOC, HB, W)
        for g in range(NPASS):
            for s in range(NS):
                out[g * CG:(g + 1) * CG, 3 * i:3 * i + DLOC,
                    HB * s:HB * s + HB, :] = yv[g, s]
    if trace:
        _CACHE["last"] = res
    return out.reshape(1, COUT, D, H, W)
